# revision 1
# baseline (speedup 1.0000x reference)
"""Trainium2 Bass kernel for nn_NodeModel (GNN message passing).

reference:
    agg = segment_sum(edge_attr, edge_index[0], num_segments=100000)   # [N, 64]
    h = concat([x, agg, u[v_indices]], axis=1)                         # [N, 256]
    out = relu(h @ W1 + b1) @ W2 + b2                                  # [N, 128]

Strategy (8 NeuronCores, SPMD, no collectives):
  - Shard nodes across cores (12500/core); shard edges by destination-node
    partition (host buckets+sorts edges by the core/block owning their row).
  - Within a core, nodes are processed in blocks of 128. Edges are sorted by
    row, grouped per block, padded to T tiles of 128 edges.
  - segment_sum on device: per 128-edge tile, build a one-hot matrix
    P[e, m] = (row_local[e] == m) with DVE/GPSIMD tensor_scalar(is_equal),
    then TensorE matmul:  aggT_hilo += ea_hilo.T @ P  accumulated in PSUM.
  - edge_attr is split hi/lo bf16 (exact to ~1e-5, same total bytes as fp32).
    The hi and lo partial sums land on PSUM partitions 0-63 / 64-127; they
    are merged for free inside the MLP by duplicating W1's agg-rows.
  - MLP runs feature-major (transposed) with fp32r matmuls, N=512 node
    groups. x and u[v_indices] are pre-transposed on host; the output is
    produced transposed and un-transposed on host.
"""

import sys

sys.path.insert(0, "/opt/trn_rl_repo")

import numpy as np
import ml_dtypes

import concourse.bass as bass
import concourse.mybir as mybir
from concourse import bacc, tile
from concourse.bass_utils import run_bass_kernel_spmd

bf16 = ml_dtypes.bfloat16

D_X, D_E, D_U = 128, 64, 64
D_HID, D_OUT = 256, 128
NB = 128  # nodes per block
WIN = 4   # 32-node one-hot windows per block

FULL_CFG = dict(n_cores=8, n_nodes=100000, npc=12500, blocks=98, group=4)

_cache = {}


def _build_nc(Tb, blocks, npad, group, n_cores=8, reps=1, opts=None):
    """Build the SPMD Bass program. Tb = per-block edge tile counts.

    reps > 1 wraps the whole computation in a hardware For_i loop — used
    only for timing (per-iteration time = delta(wall)/delta(reps), which
    cancels the host dispatch overhead)."""
    opts = dict(opts or {})
    skip_mlp = opts.get("skip_mlp", False)
    skip_edges = opts.get("skip_edges", False)
    p_bufs = opts.get("p_bufs", 20)
    ea_bufs = opts.get("ea_bufs", 5)
    gp_mod = opts.get("gp_mod", 999)
    act_mod = opts.get("act_mod", 8)  # t % act_mod == 5 -> ACT pair
    xt_ring = opts.get("xt_ring", "sync")
    Tb = list(Tb)
    offs = [0]
    for t in Tb:
        offs.append(offs[-1] + t)
    TT = offs[-1]
    max_blk_tiles = max(
        sum(Tb[b * WIN : (b + 1) * WIN]) for b in range(blocks)
    )
    nc = bacc.Bacc(
        "TRN2", target_bir_lowering=False, debug=False, num_devices=n_cores
    )
    f32, rf32, b16 = mybir.dt.float32, mybir.dt.float32r, mybir.dt.bfloat16

    # partition-major: partition = edge slot within tile, free = (block, tile, m)
    ea_in = nc.declare_dram_parameter("ea", [128, TT * 128], b16, isOutput=False)
    # idx carries a bf16 iota [128,128] packed into its first 64 f32 columns
    idx_in = nc.declare_dram_parameter("idx", [128, 64 + TT], f32, isOutput=False)
    xT_in = nc.declare_dram_parameter("xT", [128, npad], rf32, isOutput=False)
    ugT_in = nc.declare_dram_parameter("ugT", [64, npad], rf32, isOutput=False)
    # weight layouts are partition-major: [K-part, mh, M]
    w1x_in = nc.declare_dram_parameter("w1x", [128, 2, 128], rf32, isOutput=False)
    w1a_in = nc.declare_dram_parameter("w1a", [128, 2, 128], rf32, isOutput=False)
    w1u_in = nc.declare_dram_parameter("w1u", [64, 2, 128], rf32, isOutput=False)
    w2_in = nc.declare_dram_parameter("w2", [128, 2, 128], rf32, isOutput=False)
    b1_in = nc.declare_dram_parameter("b1", [128, 2], f32, isOutput=False)
    b2_in = nc.declare_dram_parameter("b2", [128, 1], f32, isOutput=False)
    outT = nc.declare_dram_parameter("outT", [128, npad], f32, isOutput=True)

    n_groups = (blocks + group - 1) // group

    with tile.TileContext(nc) as tc:
        with (
            tc.tile_pool(name="const", bufs=1) as cpool,
            tc.tile_pool(name="xt", bufs=1) as xpool,
            tc.tile_pool(name="ea", bufs=ea_bufs) as eapool,
            tc.tile_pool(name="p", bufs=p_bufs) as ppool,
            tc.tile_pool(name="hag", bufs=3) as hagpool,
            tc.tile_pool(name="ug", bufs=2) as ugpool,
            tc.tile_pool(name="h1", bufs=4) as h1pool,
            tc.tile_pool(name="outs", bufs=2) as opool,
            tc.tile_pool(name="ps_agg", bufs=4, space="PSUM") as agg_ps_pool,
            tc.tile_pool(name="ps_o1", bufs=2, space="PSUM") as o1_ps_pool,
            tc.tile_pool(name="ps_o2", bufs=2, space="PSUM") as o2_ps_pool,
        ):
          def _emit_body():
              # ---- constants / resident tensors ----
              idx_t = cpool.tile([128, 64 + TT], f32, tag="idx")
              nc.sync.dma_start(idx_t[:], idx_in[:])
              iota_ap = idx_t[:, 0:64].bitcast(b16)  # [128, 128] bf16 iota
              w1x_t = cpool.tile([128, 2, 128], rf32, tag="w1x")
              nc.sync.dma_start(w1x_t[:], w1x_in[:])
              w1a_t = cpool.tile([128, 2, 128], rf32, tag="w1a")
              nc.sync.dma_start(w1a_t[:], w1a_in[:])
              w1u_t = cpool.tile([64, 2, 128], rf32, tag="w1u")
              nc.sync.dma_start(w1u_t[:], w1u_in[:])
              w2_t = cpool.tile([128, 2, 128], rf32, tag="w2")
              nc.sync.dma_start(w2_t[:], w2_in[:])
              b1_t = cpool.tile([128, 2], f32, tag="b1")
              nc.sync.dma_start(b1_t[:], b1_in[:])
              b2_t = cpool.tile([128, 1], f32, tag="b2")
              nc.sync.dma_start(b2_t[:], b2_in[:])

              xT_t = xpool.tile([128, npad], rf32, tag="xT")
              # load x in chunks so early groups can start sooner
              xchunk = 8 * NB
              xt_eng = nc.scalar if xt_ring == "scalar" else nc.sync
              for s in range(0, npad, xchunk):
                  e = min(s + xchunk, npad)
                  xt_eng.dma_start(xT_t[:, s:e], xT_in[:, s:e])

              hag_tiles = {}
              # ---- edge scatter-add per block ----
              for b in range(blocks if not skip_edges else 0):
                  g, bi = divmod(b, group)
                  if bi == 0:
                      gw = min(group, blocks - g * group) * NB
                      hag_tiles[g] = hagpool.tile(
                          [128, group * NB], rf32, tag="hag", name=f"hag{g}"
                      )
                  Tws = Tb[b * WIN : (b + 1) * WIN]
                  o_b = offs[b * WIN]
                  Tblk = sum(Tws)
                  ea_t = eapool.tile(
                      [128, max_blk_tiles * 128], b16, tag="ea", name=f"ea{b}"
                  )
                  nc.sync.dma_start(
                      ea_t[:, : Tblk * 128],
                      ea_in[:, o_b * 128 : (o_b + Tblk) * 128],
                  )
                  if opts.get("dma_only"):
                      continue
                  agg_ps = agg_ps_pool.tile([128, NB], f32, tag="agg")
                  ti = 0
                  for w in range(WIN):
                      for t in range(Tws[w]):
                          o = o_b + ti
                          p_t = ppool.tile([128, 32], b16, tag="p")
                          idx_col = idx_t[:, 64 + o : 64 + o + 1]
                          if ti % act_mod == 5:
                              # ACT-side one-hot: P = relu(1 - (idx - iota)^2)
                              d2 = ppool.tile(
                                  [128, 32], b16, tag="d2", name=f"d2_{b}_{ti}"
                              )
                              nc.scalar.activation(
                                  out=d2[:], in_=iota_ap[:, 0:32],
                                  func=mybir.ActivationFunctionType.Square,
                                  bias=idx_col, scale=-1.0,
                              )
                              nc.scalar.activation(
                                  out=p_t[:], in_=d2[:],
                                  func=mybir.ActivationFunctionType.Relu,
                                  bias=1.0, scale=-1.0,
                              )
                          else:
                              nc.vector.tensor_scalar(
                                  out=p_t[:],
                                  in0=iota_ap[:, 0:32],
                                  scalar1=idx_col,
                                  scalar2=None,
                                  op0=mybir.AluOpType.is_equal,
                              )
                          nc.tensor.matmul(
                              agg_ps[:, 32 * w : 32 * (w + 1)],
                              ea_t[:, ti * 128 : (ti + 1) * 128],
                              p_t[:],
                              start=(t == 0),
                              stop=(t == Tws[w] - 1),
                          )
                          ti += 1
                  # move [aggT_hi ; aggT_lo] into the MLP's K-chunk staging tile
                  nc.scalar.activation(
                      out=hag_tiles[g][:, bi * NB : (bi + 1) * NB],
                      in_=agg_ps[:],
                      func=mybir.ActivationFunctionType.Copy,
                  )

              # ---- MLP per group of blocks (feature-major) ----
              no_mlp = skip_mlp or opts.get("dma_only")
              for g in range(n_groups if not no_mlp else 0):
                  s = g * group * NB
                  gw = min(group * NB, npad - s)
                  ug_t = ugpool.tile([64, group * NB], rf32, tag="ug")
                  mlp_eng = {"sync": nc.sync, "scalar": nc.scalar}[
                      opts.get("mlp_ring", "scalar")
                  ]
                  mlp_eng.dma_start(ug_t[:, :gw], ugT_in[:, s : s + gw])
                  hag = hag_tiles[g]
                  h1_list = []
                  for mh in range(2):
                      o1 = o1_ps_pool.tile([128, group * NB], f32, tag="o1")
                      nc.tensor.matmul(
                          o1[:, :gw], w1x_t[:, mh, :],
                          xT_t[:, s : s + gw],
                          start=True, stop=False,
                      )
                      nc.tensor.matmul(
                          o1[:, :gw], w1a_t[:, mh, :],
                          hag[:, :gw],
                          start=False, stop=False,
                      )
                      nc.tensor.matmul(
                          o1[:, :gw], w1u_t[:, mh, :],
                          ug_t[:, :gw],
                          start=False, stop=True,
                      )
                      h1 = h1pool.tile([128, group * NB], rf32, tag="h1")
                      nc.scalar.activation(
                          out=h1[:, :gw], in_=o1[:, :gw],
                          func=mybir.ActivationFunctionType.Relu,
                          bias=b1_t[:, mh : mh + 1],
                      )
                      h1_list.append(h1)
                  o2 = o2_ps_pool.tile([128, group * NB], f32, tag="o2")
                  for kh in range(2):
                      nc.tensor.matmul(
                          o2[:, :gw], w2_t[:, kh, :],
                          h1_list[kh][:, :gw],
                          start=(kh == 0), stop=(kh == 1),
                      )
                  out_t = opool.tile([128, group * NB], f32, tag="outs")
                  nc.scalar.activation(
                      out=out_t[:, :gw], in_=o2[:, :gw],
                      func=mybir.ActivationFunctionType.Identity,
                      bias=b2_t[:],
                  )
                  mlp_eng.dma_start(outT[:, s : s + gw], out_t[:, :gw])

          if reps == 1:
              _emit_body()
          else:
              with tc.For_i(0, reps, 1):
                  _emit_body()

    nc.compile()
    return nc


def _pack_inputs(x, edge_index, edge_attr, u, v_indices, W1, b1, W2, b2, cfg):
    """Host-side sharding: bucket + sort edges by destination node partition."""
    n_cores, npc, blocks = cfg["n_cores"], cfg["npc"], cfg["blocks"]
    n_nodes = cfg["n_nodes"]
    npad = blocks * NB
    row = np.asarray(edge_index[0], dtype=np.int64)
    ea = np.ascontiguousarray(np.asarray(edge_attr, dtype=np.float32))
    x = np.asarray(x, dtype=np.float32)
    u = np.asarray(u, dtype=np.float32)
    v_indices = np.asarray(v_indices, dtype=np.int64)
    W1 = np.asarray(W1, dtype=np.float32)
    W2 = np.asarray(W2, dtype=np.float32)
    b1 = np.asarray(b1, dtype=np.float32)
    b2 = np.asarray(b2, dtype=np.float32)
    d_e = ea.shape[1]

    order = np.argsort(row, kind="stable")
    row_s = row[order]
    ea_s = ea[order]
    hi = ea_s.astype(bf16)
    lo = (ea_s - hi.astype(np.float32)).astype(bf16)
    ea_hilo = np.concatenate([hi, lo], axis=1)  # [E, 2*d_e] bf16

    # window boundaries: core c window i covers nodes [npc*c + 32*i, +32),
    # clipped to the core's node range (WIN windows per 128-node block).
    nwin = blocks * WIN
    bases = (npc * np.arange(n_cores)[:, None] + 32 * np.arange(nwin)[None, :]).ravel()
    core_hi = (npc * (1 + np.arange(n_cores))[:, None]).repeat(nwin, 1).ravel()
    starts = np.searchsorted(row_s, np.minimum(bases, core_hi), side="left")
    ends = np.searchsorted(row_s, np.minimum(bases + 32, core_hi), side="left")
    cnts = (ends - starts).reshape(n_cores, nwin)
    Tb = np.maximum(1, (cnts.max(axis=0) + 127) // 128).astype(int)  # [nwin]
    offs = np.concatenate([[0], np.cumsum(Tb)])
    TT = int(offs[-1])

    # ea layout: [core][partition=slot%128, (offs[b] + tile)*128 + m]
    # Vectorized scatter: edge k of block b (rank r within the block) lands in
    # slot offs[b]*128 + r of its core's slot array.
    ea_pack = np.empty((n_cores, 128, TT * 128), dtype=bf16)
    idx_pack = np.empty((n_cores, 128, TT), dtype=np.float32)
    starts2 = starts.reshape(n_cores, nwin)
    for c in range(n_cores):
        cs, ce = starts2[c, 0], ends.reshape(n_cores, nwin)[c, -1]
        wb = np.repeat(np.arange(nwin), cnts[c])            # window id per edge
        rank = np.arange(ce - cs) - np.repeat(starts2[c] - cs, cnts[c])
        slot = offs[wb] * 128 + rank
        coreslots = np.zeros((TT * 128, 2 * d_e), dtype=bf16)
        coreslots[slot] = ea_hilo[cs:ce]
        ea_pack[c] = (
            coreslots.reshape(TT, 128, 2 * d_e)
            .transpose(1, 0, 2)
            .reshape(128, TT * 128)
        )
        ivals = np.zeros(TT * 128, dtype=np.float32)
        ivals[slot] = (row_s[cs:ce] - (npc * c + 32 * wb)).astype(np.float32)
        idx_pack[c] = ivals.reshape(TT, 128).T

    iota = np.broadcast_to(np.arange(128, dtype=np.float32), (128, 128)).astype(bf16)
    iota_f32 = np.ascontiguousarray(iota).view(np.float32)  # [128, 64]
    uT = u.T  # [d_u, n_graphs]

    # weights, partition-major [K, mh, M]
    w1x = np.ascontiguousarray(W1[:D_X].reshape(D_X, 2, 128))
    w1a_single = W1[D_X : D_X + d_e]                       # [64, 256]
    w1a_dup = np.concatenate([w1a_single, w1a_single], 0)  # [128, 256] hi|lo dup
    w1a = np.ascontiguousarray(w1a_dup.reshape(128, 2, 128))
    w1u = np.ascontiguousarray(W1[D_X + d_e :].reshape(D_U, 2, 128))
    w2 = np.ascontiguousarray(W2.reshape(2, 128, D_OUT).transpose(1, 0, 2))
    b1p = np.ascontiguousarray(b1.reshape(2, 128).T)
    b2p = np.ascontiguousarray(b2.reshape(128, 1))

    in_maps = []
    for c in range(n_cores):
        lo_n, hi_n = npc * c, min(npc * (c + 1), n_nodes)
        xT = np.zeros((D_X, npad), dtype=np.float32)
        xT[:, : hi_n - lo_n] = x[lo_n:hi_n].T
        ugT = np.zeros((D_U, npad), dtype=np.float32)
        ugT[:, : hi_n - lo_n] = uT[:, v_indices[lo_n:hi_n]]
        in_maps.append({
            "ea": ea_pack[c],
            "idx": np.concatenate([iota_f32, idx_pack[c]], axis=1),
            "xT": xT,
            "ugT": ugT,
            "w1x": w1x,
            "w1a": w1a,
            "w1u": w1u,
            "w2": w2,
            "b1": b1p,
            "b2": b2p,
        })
    return in_maps, tuple(int(t) for t in Tb)


def _run(inputs, cfg, trace=False, reps=1):
    in_maps, T = _pack_inputs(
        inputs["x"], inputs["edge_index"], inputs["edge_attr"], inputs["u"],
        inputs["v_indices"], inputs["W1"], inputs["b1"], inputs["W2"],
        inputs["b2"], cfg,
    )
    key = (T, cfg["blocks"], cfg["group"], reps)
    if key not in _cache:
        _cache[key] = _build_nc(
            T, cfg["blocks"], cfg["blocks"] * NB, cfg["group"], reps=reps
        )
    nc = _cache[key]
    res = run_bass_kernel_spmd(nc, in_maps, list(range(cfg["n_cores"])), trace=trace)
    n_nodes, npc = cfg["n_nodes"], cfg["npc"]
    out = np.empty((n_nodes, D_OUT), dtype=np.float32)
    for c in range(cfg["n_cores"]):
        lo_n, hi_n = npc * c, min(npc * (c + 1), n_nodes)
        out[lo_n:hi_n] = res.results[c]["outT"].T[: hi_n - lo_n]
    _run.last_results = res
    return out


def kernel(x, edge_index, edge_attr, u, v_indices, W1, b1, W2, b2):
    inputs = dict(x=x, edge_index=edge_index, edge_attr=edge_attr, u=u,
                  v_indices=v_indices, W1=W1, b1=b1, W2=W2, b2=b2)
    return _run(inputs, FULL_CFG)



# revision 2
# speedup vs baseline: 2.2017x; 2.2017x over previous
"""Trainium2 Bass kernel for nn_NodeModel (GNN message passing).

reference:
    agg = segment_sum(edge_attr, edge_index[0], num_segments=100000)   # [N, 64]
    h = concat([x, agg, u[v_indices]], axis=1)                         # [N, 256]
    out = relu(h @ W1 + b1) @ W2 + b2                                  # [N, 128]

Strategy (8 NeuronCores, SPMD, no collectives):
  - Nodes are assigned to (core, window-of-32) slots by a degree-balanced
    snake deal: nodes sorted by degree are dealt round-robin (alternating
    direction) across all 3200 (core, window) bins, so every bin holds ~1/3200
    of all edges. Edges follow their destination node's bin. This keeps the
    per-window 128-edge tile counts almost equal to the zero-padding-free
    minimum (output is un-permuted on the host at the end).
  - edge_attr is sent as bf16 (hi half only; ~0.1% rel err, well under the
    2e-2 gate) => half the dominant HBM traffic.
  - segment_sum on device: per 128-edge tile, one-hot P[e, m] = (idx[e] == m)
    built for ALL tiles of a 128-node block in ONE DVE tensor_tensor using
    stride-0 broadcast APs, then TensorE matmuls aggT += ea.T @ P per window
    accumulated in PSUM.
  - MLP feature-major in bf16 (x, u-gather pre-transposed/gathered on host).
    agg (64 rows) and u-gather (64 rows) share one 128-partition tile, so
    h@W1 is 2 matmuls per 128-col half instead of 3. ReLU is split between
    ACT (mh=0) and DVE (mh=1) to balance engines.
"""

import sys

sys.path.insert(0, "/opt/trn_rl_repo")

import numpy as np
import ml_dtypes

import concourse.bass as bass
import concourse.mybir as mybir
from concourse import bacc, tile
from concourse.bass_utils import run_bass_kernel_spmd

bf16 = ml_dtypes.bfloat16

D_X, D_E, D_U = 128, 64, 64
D_HID, D_OUT = 256, 128
NB = 128   # nodes per block
WSZ = 32   # nodes per one-hot window
WIN = NB // WSZ

FULL_CFG = dict(
    n_cores=8, n_nodes=100000, blocks=100, group=4
)  # 12800 node slots/core

_cache = {}


def _build_nc(Tb, blocks, npad, group, n_cores=8, reps=1, opts=None):
    """Build the SPMD Bass program. Tb = per-window edge tile counts
    (shared across cores; windows are 32 nodes, WIN windows per block).

    reps > 1 wraps the computation in a hardware For_i loop — used only
    for timing (per-iter time = delta(wall)/delta(reps), cancelling the
    host dispatch overhead)."""
    opts = dict(opts or {})
    ea_bufs = opts.get("ea_bufs", 5)
    p_bufs = opts.get("p_bufs", 4)
    Tb = list(Tb)
    offs = [0]
    for t in Tb:
        offs.append(offs[-1] + t)
    TT = offs[-1]
    max_blk_tiles = max(
        sum(Tb[b * WIN : (b + 1) * WIN]) for b in range(blocks)
    )
    nc = bacc.Bacc(
        "TRN2", target_bir_lowering=False, debug=False, num_devices=n_cores
    )
    f32, b16 = mybir.dt.float32, mybir.dt.bfloat16

    GW = group * NB  # nodes per MLP group

    # partition-major layouts; [K, mh, M] for weights
    ea_in = nc.declare_dram_parameter("ea", [128, TT * 64], b16, isOutput=False)
    idx_in = nc.declare_dram_parameter("idx", [128, TT], b16, isOutput=False)
    iota_in = nc.declare_dram_parameter("iota", [128, WSZ], b16, isOutput=False)
    xT_in = nc.declare_dram_parameter("xT", [128, npad], b16, isOutput=False)
    ugT_in = nc.declare_dram_parameter("ugT", [64, npad], b16, isOutput=False)
    w1x_in = nc.declare_dram_parameter("w1x", [128, 2, 128], b16, isOutput=False)
    w1au_in = nc.declare_dram_parameter("w1au", [128, 2, 128], b16, isOutput=False)
    w2_in = nc.declare_dram_parameter("w2", [128, 2, 128], b16, isOutput=False)
    b1_in = nc.declare_dram_parameter("b1", [128, 2], f32, isOutput=False)
    b2_in = nc.declare_dram_parameter("b2", [128, 1], f32, isOutput=False)
    outT = nc.declare_dram_parameter("outT", [128, npad], f32, isOutput=True)

    n_groups = (blocks + group - 1) // group

    with tile.TileContext(nc) as tc:
        with (
            tc.tile_pool(name="const", bufs=1) as cpool,
            tc.tile_pool(name="x", bufs=3) as xpool,
            tc.tile_pool(name="ea", bufs=ea_bufs) as eapool,
            tc.tile_pool(name="p", bufs=p_bufs) as ppool,
            tc.tile_pool(name="aug", bufs=3) as augpool,
            tc.tile_pool(name="h1", bufs=4) as h1pool,
            tc.tile_pool(name="outs", bufs=3) as opool,
            tc.tile_pool(name="ps_agg", bufs=4, space="PSUM") as agg_ps_pool,
            tc.tile_pool(name="ps_o1", bufs=2, space="PSUM") as o1_ps_pool,
            tc.tile_pool(name="ps_o2", bufs=2, space="PSUM") as o2_ps_pool,
        ):
          def _emit_body():
              # ---- constants ----
              idx_t = cpool.tile([128, TT], b16, tag="idx")
              nc.sync.dma_start(idx_t[:], idx_in[:])
              iota_t = cpool.tile([128, WSZ], b16, tag="iota")
              nc.sync.dma_start(iota_t[:], iota_in[:])
              w1x_t = cpool.tile([128, 2, 128], b16, tag="w1x")
              nc.sync.dma_start(w1x_t[:], w1x_in[:])
              w1au_t = cpool.tile([128, 2, 128], b16, tag="w1au")
              nc.sync.dma_start(w1au_t[:], w1au_in[:])
              w2_t = cpool.tile([128, 2, 128], b16, tag="w2")
              nc.sync.dma_start(w2_t[:], w2_in[:])
              b1_t = cpool.tile([128, 2], f32, tag="b1")
              nc.sync.dma_start(b1_t[:], b1_in[:])
              b2_t = cpool.tile([128, 1], f32, tag="b2")
              nc.sync.dma_start(b2_t[:], b2_in[:])

              aug_tiles = {}
              for b in range(blocks):
                  g, bi = divmod(b, group)
                  if bi == 0:
                      aug_tiles[g] = augpool.tile(
                          [128, GW], b16, tag="aug", name=f"aug{g}"
                      )
                  Tws = Tb[b * WIN : (b + 1) * WIN]
                  o_b = offs[b * WIN]
                  Tblk = sum(Tws)
                  # ---- edge tiles for this block ----
                  ea_t = eapool.tile(
                      [128, max_blk_tiles * 64], b16, tag="ea", name=f"ea{b}"
                  )
                  nc.sync.dma_start(
                      ea_t[:, : Tblk * 64],
                      ea_in[:, o_b * 64 : (o_b + Tblk) * 64],
                  )
                  # one-hot for all tiles of the block in ONE DVE op
                  p_t = ppool.tile([128, max_blk_tiles, WSZ], b16, tag="p")
                  nc.vector.tensor_tensor(
                      out=p_t[:, 0:Tblk, :],
                      in0=idx_t[:, o_b : o_b + Tblk]
                      .unsqueeze(2)
                      .broadcast_to([128, Tblk, WSZ]),
                      in1=iota_t[:, 0:WSZ]
                      .unsqueeze(1)
                      .broadcast_to([128, Tblk, WSZ]),
                      op=mybir.AluOpType.is_equal,
                  )
                  # segment-sum via matmul per tile, accumulated per window
                  agg_ps = agg_ps_pool.tile([64, NB], f32, tag="agg")
                  ti = 0
                  for w in range(WIN):
                      for t in range(Tws[w]):
                          nc.tensor.matmul(
                              agg_ps[:, WSZ * w : WSZ * (w + 1)],
                              ea_t[:, ti * 64 : (ti + 1) * 64],
                              p_t[:, ti, :],
                              start=(t == 0),
                              stop=(t == Tws[w] - 1),
                          )
                          ti += 1
                  # stage aggT into the group's combined [agg; ug] tile
                  nc.scalar.activation(
                      out=aug_tiles[g][0:64, bi * NB : (bi + 1) * NB],
                      in_=agg_ps[:],
                      func=mybir.ActivationFunctionType.Copy,
                  )

                  # ---- MLP once the group's 4 blocks are aggregated ----
                  if bi == group - 1:
                      s = g * GW
                      aug = aug_tiles.pop(g)
                      nc.scalar.dma_start(
                          aug[64:128, :], ugT_in[:, s : s + GW]
                      )
                      x_t = xpool.tile([128, GW], b16, tag="x")
                      nc.sync.dma_start(x_t[:], xT_in[:, s : s + GW])
                      h1_list = []
                      for mh in range(2):
                          o1 = o1_ps_pool.tile([128, GW], f32, tag="o1")
                          nc.tensor.matmul(
                              o1[:], w1x_t[:, mh, :], x_t[:],
                              start=True, stop=False,
                          )
                          nc.tensor.matmul(
                              o1[:], w1au_t[:, mh, :], aug[:],
                              start=False, stop=True,
                          )
                          h1 = h1pool.tile([128, GW], b16, tag="h1")
                          if mh == 0:
                              nc.scalar.activation(
                                  out=h1[:], in_=o1[:],
                                  func=mybir.ActivationFunctionType.Relu,
                                  bias=b1_t[:, 0:1],
                              )
                          else:
                              nc.vector.tensor_scalar(
                                  out=h1[:], in0=o1[:],
                                  scalar1=b1_t[:, 1:2], scalar2=0.0,
                                  op0=mybir.AluOpType.add,
                                  op1=mybir.AluOpType.max,
                              )
                          h1_list.append(h1)
                      o2 = o2_ps_pool.tile([128, GW], f32, tag="o2")
                      for kh in range(2):
                          nc.tensor.matmul(
                              o2[:], w2_t[:, kh, :], h1_list[kh][:],
                              start=(kh == 0), stop=(kh == 1),
                          )
                      out_t = opool.tile([128, GW], f32, tag="outs")
                      nc.scalar.activation(
                          out=out_t[:], in_=o2[:],
                          func=mybir.ActivationFunctionType.Identity,
                          bias=b2_t[:],
                      )
                      nc.scalar.dma_start(outT[:, s : s + GW], out_t[:])

          if reps == 1:
              _emit_body()
          else:
              with tc.For_i(0, reps, 1):
                  _emit_body()

    nc.compile()
    return nc


def _pack_inputs(x, edge_index, edge_attr, u, v_indices, W1, b1, W2, b2, cfg):
    """Host-side sharding: degree-balanced node permutation + edge packing."""
    n_cores, blocks = cfg["n_cores"], cfg["blocks"]
    n_nodes = cfg["n_nodes"]
    npad = blocks * NB
    nwin = npad // WSZ           # windows per core
    nbins = n_cores * nwin       # (core, window) bins
    nslots = nbins * WSZ
    row = np.asarray(edge_index[0], dtype=np.int64)
    ea = np.asarray(edge_attr, dtype=np.float32)
    x = np.asarray(x, dtype=np.float32)
    u = np.asarray(u, dtype=np.float32)
    v_indices = np.asarray(v_indices, dtype=np.int64)
    W1 = np.asarray(W1, dtype=np.float32)
    W2 = np.asarray(W2, dtype=np.float32)
    b1 = np.asarray(b1, dtype=np.float32)
    b2 = np.asarray(b2, dtype=np.float32)
    d_e = ea.shape[1]

    # ---- snake-deal nodes (sorted by degree desc) across bins ----
    deg = np.bincount(row, minlength=n_nodes)
    order = np.argsort(-deg, kind="stable")          # high degree first
    node_core = np.empty(n_nodes, np.int32)
    node_win = np.empty(n_nodes, np.int32)
    node_off = np.empty(n_nodes, np.int32)
    pos = np.arange(nslots)
    rounds, cols = pos // nbins, pos % nbins
    bins = np.where(rounds % 2 == 0, cols, nbins - 1 - cols)
    rb, bb = rounds[:n_nodes], bins[:n_nodes]
    node_core[order] = (bb // nwin).astype(np.int32)
    node_win[order] = (bb % nwin).astype(np.int32)
    node_off[order] = rb.astype(np.int32)
    node_plocal = node_win * WSZ + node_off          # slot within core

    # ---- edge buckets ----
    ec = node_core[row]
    ew = node_win[row]
    em = node_off[row]
    key = ec.astype(np.int64) * nwin + ew
    cnt = np.bincount(key, minlength=nbins).reshape(n_cores, nwin)
    Tb = np.maximum(1, (cnt.max(axis=0) + 127) // 128).astype(int)  # [nwin]
    offs = np.concatenate([[0], np.cumsum(Tb)])
    TT = int(offs[-1])

    order_e = np.argsort(key, kind="stable")
    key_s = key[order_e]
    cnt_flat = np.bincount(key_s, minlength=nbins)
    starts_flat = np.concatenate([[0], np.cumsum(cnt_flat)])[:-1]
    rank = np.arange(len(key_s)) - starts_flat[key_s]
    ew_s = ew[order_e]
    slot = offs[ew_s] * 128 + rank                   # within-core slot
    ec_s = ec[order_e]
    em_s = em[order_e].astype(np.float32)
    ea_hi = ea[order_e].astype(bf16)

    ea_pack = np.empty((n_cores, 128, TT * 64), dtype=bf16)
    idx_pack = np.empty((n_cores, 128, TT), dtype=bf16)
    for c in range(n_cores):
        m = ec_s == c
        coreslots = np.zeros((TT * 128, d_e), dtype=bf16)
        coreslots[slot[m]] = ea_hi[m]
        ea_pack[c] = (
            coreslots.reshape(TT, 128, d_e).transpose(1, 0, 2).reshape(128, -1)
        )
        ivals = np.zeros(TT * 128, dtype=np.float32)
        ivals[slot[m]] = em_s[m]
        idx_pack[c] = ivals.reshape(TT, 128).T.astype(bf16)

    iota = np.broadcast_to(
        np.arange(WSZ, dtype=np.float32), (128, WSZ)
    ).astype(bf16)
    uT = u.T  # [d_u, n_graphs]

    w1x = np.ascontiguousarray(W1[:D_X].reshape(D_X, 2, 128)).astype(bf16)
    w1au = np.ascontiguousarray(W1[D_X:].reshape(128, 2, 128)).astype(bf16)
    w2 = np.ascontiguousarray(
        W2.reshape(2, 128, D_OUT).transpose(1, 0, 2)
    ).astype(bf16)
    b1p = np.ascontiguousarray(b1.reshape(2, 128).T)
    b2p = np.ascontiguousarray(b2.reshape(128, 1))

    in_maps = []
    for c in range(n_cores):
        sel = node_core == c
        pl = node_plocal[sel]
        xT = np.zeros((D_X, npad), dtype=bf16)
        xT[:, pl] = x[sel].T.astype(bf16)
        ugT = np.zeros((D_U, npad), dtype=bf16)
        ugT[:, pl] = uT[:, v_indices[sel]].astype(bf16)
        in_maps.append({
            "ea": ea_pack[c],
            "idx": idx_pack[c],
            "iota": iota,
            "xT": xT,
            "ugT": ugT,
            "w1x": w1x,
            "w1au": w1au,
            "w2": w2,
            "b1": b1p,
            "b2": b2p,
        })
    unperm = (node_core, node_plocal)
    return in_maps, tuple(int(t) for t in Tb), unperm


def _unpack_output(res_per_core, unperm, cfg):
    node_core, node_plocal = unperm
    n_nodes = cfg["n_nodes"]
    out = np.empty((n_nodes, D_OUT), dtype=np.float32)
    for c in range(cfg["n_cores"]):
        sel = node_core == c
        out[sel] = res_per_core[c].T[node_plocal[sel]]
    return out


def _run(inputs, cfg, reps=1):
    in_maps, T, unperm = _pack_inputs(
        inputs["x"], inputs["edge_index"], inputs["edge_attr"], inputs["u"],
        inputs["v_indices"], inputs["W1"], inputs["b1"], inputs["W2"],
        inputs["b2"], cfg,
    )
    key = (T, cfg["blocks"], cfg["group"], reps)
    if key not in _cache:
        _cache[key] = _build_nc(
            T, cfg["blocks"], cfg["blocks"] * NB, cfg["group"],
            n_cores=cfg["n_cores"], reps=reps,
        )
    nc = _cache[key]
    res = run_bass_kernel_spmd(nc, in_maps, list(range(cfg["n_cores"])))
    return _unpack_output(
        [res.results[c]["outT"] for c in range(cfg["n_cores"])], unperm, cfg
    )


def kernel(x, edge_index, edge_attr, u, v_indices, W1, b1, W2, b2):
    inputs = dict(x=x, edge_index=edge_index, edge_attr=edge_attr, u=u,
                  v_indices=v_indices, W1=W1, b1=b1, W2=W2, b2=b2)
    return _run(inputs, FULL_CFG)
